# revision 1
# baseline (speedup 1.0000x reference)
"""Trainium2 Bass kernel for nn_Attention2 (attention-gated blend of Z_l/Z_g).

Reference math:
    Q     = Z_o @ W.T + b                      # [N, 512]
    att_l = Q @ colsum(Z_l)                    # [N]
    att_g = Q @ colsum(Z_g)                    # [N]
    att   = softmax([att_l, att_g], axis=1)    # [N, 2]
    out   = Z_l * att[:, 0:1] + Z_g * att[:, 1:2]

Only d = att_l - att_g matters (2-way softmax == sigmoid), and it folds:
    s = colsum(Z_l) - colsum(Z_g)              # [512]
    u = W.T @ s                                # [512]
    c = b . s                                  # scalar
    d = Z_o @ u + c                            # [N]
    out = Z_l * sigmoid(d) + Z_g * sigmoid(-d)

This removes the O(N*512*512) matmul; the kernel is HBM-bound.

Sharding: data-parallel over rows, N/8 rows per core. Two SPMD launches:
  A: per-core partial s = colsum(Z_l) - colsum(Z_g) via TensorE ones-matmul
     accumulation into one PSUM row -> [1, 512] per core.
  host: reduce partials (f64), u = W.T @ s, c = b . s  (tiny: 512x512 matvec)
  B: streams Z_o/Z_l/Z_g row tiles; d = rowwise_dot(Z_o, u) (+c via sigmoid
     bias); out = Z_l*sig(d) + Z_g*sig(-d); writes out.
"""

import numpy as np

import concourse.bacc as bacc
import concourse.mybir as mybir
import concourse.tile as tile
from concourse.bass_utils import run_bass_kernel_spmd

N_CORES = 8
N_TOTAL = 100000
CH = 512
SHARD = N_TOTAL // N_CORES  # 12500
P = 128


def build_nc_a(shard=SHARD, n_cores=N_CORES, bufs=6, rep_loop=1):
    import contextlib

    f32 = mybir.dt.float32
    nc = bacc.Bacc(
        "TRN2",
        target_bir_lowering=False,
        debug=False,
        enable_asserts=False,
        num_devices=n_cores,
    )
    zl_d = nc.dram_tensor("Z_l", [shard, CH], f32, kind="ExternalInput")
    zg_d = nc.dram_tensor("Z_g", [shard, CH], f32, kind="ExternalInput")
    s_d = nc.dram_tensor("s_part", [1, CH], f32, kind="ExternalOutput")
    n_tiles = (shard + P - 1) // P
    with tile.TileContext(nc) as tc:
        with (
            tc.tile_pool(name="singles", bufs=1) as singles,
            tc.tile_pool(name="psum", bufs=1, space="PSUM") as psum,
            tc.tile_pool(name="p1", bufs=bufs) as p1,
        ):
            ones_col = singles.tile([P, 1], f32)
            neg_ones_col = singles.tile([P, 1], f32)
            nc.vector.memset(ones_col[:], 1.0)
            nc.vector.memset(neg_ones_col[:], -1.0)
            rep_ctx = (
                tc.For_i(0, rep_loop, 1) if rep_loop > 1 else contextlib.nullcontext()
            )
            with rep_ctx:
                ps_s = psum.tile([1, CH], f32)
                for i in range(n_tiles):
                    r0 = i * P
                    R = min(P, shard - r0)
                    zl = p1.tile([P, CH], f32, tag="zl")
                    zg = p1.tile([P, CH], f32, tag="zg")
                    nc.sync.dma_start(zl[:R], zl_d[r0 : r0 + R])
                    nc.sync.dma_start(zg[:R], zg_d[r0 : r0 + R])
                    nc.tensor.matmul(
                        ps_s[:], ones_col[:R], zl[:R], start=(i == 0), stop=False
                    )
                    nc.tensor.matmul(
                        ps_s[:],
                        neg_ones_col[:R],
                        zg[:R],
                        start=False,
                        stop=(i == n_tiles - 1),
                    )
                s_sb = singles.tile([1, CH], f32)
                nc.vector.tensor_copy(s_sb[:], ps_s[:])
                nc.sync.dma_start(s_d[:, :], s_sb[:])
    nc.compile()
    return nc


def build_nc_b(shard=SHARD, n_cores=N_CORES, bufs=6, rep_loop=1, add_on_gpsimd=False):
    """Phase 2, core-ISA ops only; u_b/c_b/ncb arrive pre-broadcast from host.

    rep_loop > 1 wraps the tile loop in a device-side For loop that re-runs
    the whole pass (same data) -- used only for timing measurements.
    """
    import contextlib

    f32 = mybir.dt.float32
    add = mybir.AluOpType.add
    AF = mybir.ActivationFunctionType
    nc = bacc.Bacc(
        "TRN2",
        target_bir_lowering=False,
        debug=False,
        enable_asserts=False,
        num_devices=n_cores,
    )
    zo_d = nc.dram_tensor("Z_o", [shard, CH], f32, kind="ExternalInput")
    zl_d = nc.dram_tensor("Z_l", [shard, CH], f32, kind="ExternalInput")
    zg_d = nc.dram_tensor("Z_g", [shard, CH], f32, kind="ExternalInput")
    ub_d = nc.dram_tensor("u_b", [P, CH], f32, kind="ExternalInput")
    cb_d = nc.dram_tensor("c_b", [P, 32], f32, kind="ExternalInput")
    ncb_d = nc.dram_tensor("ncb", [P, 32], f32, kind="ExternalInput")
    out_d = nc.dram_tensor("out", [shard, CH], f32, kind="ExternalOutput")
    n_tiles = (shard + P - 1) // P
    with tile.TileContext(nc) as tc:
        with (
            tc.tile_pool(name="singles", bufs=1) as singles,
            tc.tile_pool(name="p2", bufs=bufs) as p2,
            tc.tile_pool(name="small2", bufs=bufs) as small2,
        ):
            u_b = singles.tile([P, CH], f32)
            nc.sync.dma_start(u_b[:], ub_d[:, :])
            c_b = singles.tile([P, 32], f32)
            nc.sync.dma_start(c_b[:], cb_d[:, :])
            ncb = singles.tile([P, 32], f32)
            nc.sync.dma_start(ncb[:], ncb_d[:, :])
            rep_ctx = (
                tc.For_i(0, rep_loop, 1) if rep_loop > 1 else contextlib.nullcontext()
            )
            with rep_ctx:
                _body_b(nc, tc, p2, small2, zo_d, zl_d, zg_d, out_d, u_b, c_b, ncb,
                        n_tiles, shard, add_on_gpsimd)
    nc.compile()
    return nc


def _body_b(nc, tc, p2, small2, zo_d, zl_d, zg_d, out_d, u_b, c_b, ncb, n_tiles,
            shard, add_on_gpsimd=False):
    f32 = mybir.dt.float32
    add = mybir.AluOpType.add
    AF = mybir.ActivationFunctionType
    if True:
        for i in range(n_tiles):
                r0 = i * P
                R = min(P, shard - r0)
                zo = p2.tile([P, CH], f32, tag="zo")
                zl = p2.tile([P, CH], f32, tag="zl2")
                zg = p2.tile([P, CH], f32, tag="zg2")
                nc.sync.dma_start(zo[:R], zo_d[r0 : r0 + R])
                nc.sync.dma_start(zl[:R], zl_d[r0 : r0 + R])
                nc.sync.dma_start(zg[:R], zg_d[r0 : r0 + R])
                prod = p2.tile([P, CH], f32, tag="prod")
                nc.vector.tensor_mul(prod[:R], zo[:R], u_b[:R])
                d_t = small2.tile([P, 1], f32, tag="d")
                nc.vector.tensor_reduce(
                    d_t[:R], prod[:R], axis=mybir.AxisListType.X, op=add
                )
                frac = small2.tile([P, 1], f32, tag="fr")
                om = small2.tile([P, 1], f32, tag="om")
                nc.scalar.activation(
                    frac[:R], d_t[:R], AF.Sigmoid, bias=c_b[:R, 0:1], scale=1.0
                )
                nc.scalar.activation(
                    om[:R], d_t[:R], AF.Sigmoid, bias=ncb[:R, 0:1], scale=-1.0
                )
                t1 = p2.tile([P, CH], f32, tag="t1")
                nc.scalar.mul(t1[:R], zl[:R], frac[:R, 0:1])
                t2 = p2.tile([P, CH], f32, tag="t2")
                nc.scalar.mul(t2[:R], zg[:R], om[:R, 0:1])
                ot = p2.tile([P, CH], f32, tag="ot")
                add_eng = nc.gpsimd if add_on_gpsimd else nc.vector
                add_eng.tensor_add(ot[:R], t1[:R], t2[:R])
                nc.sync.dma_start(out_d[r0 : r0 + R], ot[:R])


_CACHE = {}


def kernel(Z_o, Z_l, Z_g, W, b):
    Z_o = np.ascontiguousarray(np.asarray(Z_o, dtype=np.float32))
    Z_l = np.ascontiguousarray(np.asarray(Z_l, dtype=np.float32))
    Z_g = np.ascontiguousarray(np.asarray(Z_g, dtype=np.float32))
    W = np.ascontiguousarray(np.asarray(W, dtype=np.float32))
    b = np.ascontiguousarray(np.asarray(b, dtype=np.float32))
    if "a" not in _CACHE:
        _CACHE["a"] = build_nc_a()
        _CACHE["b"] = build_nc_b()
    nc_a, nc_b = _CACHE["a"], _CACHE["b"]
    sh = SHARD
    maps_a = [
        {"Z_l": Z_l[i * sh : (i + 1) * sh], "Z_g": Z_g[i * sh : (i + 1) * sh]}
        for i in range(N_CORES)
    ]
    res_a = run_bass_kernel_spmd(nc_a, maps_a, core_ids=list(range(N_CORES)))
    s = sum(r["s_part"][0].astype(np.float64) for r in res_a.results)
    u = (W.astype(np.float64).T @ s).astype(np.float32)
    c = np.float32(b.astype(np.float64) @ s)
    u_b = np.ascontiguousarray(np.broadcast_to(u, (P, CH)))
    c_b = np.full((P, 32), c, dtype=np.float32)
    ncb = -c_b
    maps_b = [
        {
            "Z_o": Z_o[i * sh : (i + 1) * sh],
            "Z_l": Z_l[i * sh : (i + 1) * sh],
            "Z_g": Z_g[i * sh : (i + 1) * sh],
            "u_b": u_b,
            "c_b": c_b,
            "ncb": ncb,
        }
        for i in range(N_CORES)
    ]
    res_b = run_bass_kernel_spmd(nc_b, maps_b, core_ids=list(range(N_CORES)))
    return np.concatenate([r["out"] for r in res_b.results], axis=0)



# revision 6
# speedup vs baseline: 4.9611x; 4.9611x over previous
"""Trainium2 Bass kernel for nn_Attention2 (attention-gated blend of Z_l/Z_g).

Reference math:
    Q     = Z_o @ W.T + b                      # [N, 512]
    att_l = Q @ colsum(Z_l)                    # [N]
    att_g = Q @ colsum(Z_g)                    # [N]
    att   = softmax([att_l, att_g], axis=1)    # [N, 2]
    out   = Z_l * att[:, 0:1] + Z_g * att[:, 1:2]

Only d = att_l - att_g matters (2-way softmax == sigmoid), and it folds:
    s = colsum(Z_l) - colsum(Z_g)              # [512]
    u = W.T @ s                                # [512]
    c = b . s                                  # scalar
    d = Z_o @ u + c                            # [N]
    out = Z_g + sigmoid(d) * (Z_l - Z_g)

Fused single-launch SPMD design (8 cores, rows sharded):
  Stage 1: stream Z_l/Z_g (fp16) row tiles; accumulate s_partial in PSUM via
    TensorE (+1/-1 ones matmuls); cache zd = Z_l - Z_g (all tiles) and Z_g
    (most tiles) in SBUF.
  AllReduce(add) of s_partial [1,512] f32 across the 8 cores (DRAM bounce).
  Interlude (on-device): u = W.T @ s via 4 PSUM-accumulated matmuls; split
    u into fp16 (hi, lo) pair so the stage-2 fp16 matmul keeps f32 accuracy
    of u; c = b.s; broadcast c across partitions.
  Stage 2: stream Z_o TRANSPOSED (fp16, [512, shard]); per 128-row tile
    d = zoT_chunk^T @ u_hi + zoT_chunk^T @ u_lo accumulated in PSUM [R,1];
    p = sigmoid(d + c) on ScalarE (reads PSUM); out = (zd * p) + zg in one
    DVE scalar_tensor_tensor; DMA out (fp16).
Host: downcast/transpose inputs, upcast output; tiny work only.

Precision: fp16 inputs give rel err ~1.3e-2 vs the f32 reference (gate
2e-2): colsum-of-rounded-inputs and Z_o rounding each contribute ~9e-3
through the sigmoid boundary rows; blend/output rounding is negligible.
"""

import numpy as np

import concourse.bacc as bacc
import concourse.mybir as mybir
import concourse.tile as tile
from concourse.bass_utils import run_bass_kernel_spmd

N_CORES = 8
N_TOTAL = 100000
CH = 512
SHARD = N_TOTAL // N_CORES  # 12500
P = 128
N_TILES = (SHARD + P - 1) // P  # 98 (last tile 84 rows)
GROUP_ROWS = 512
N_GROUPS = (SHARD + GROUP_ROWS - 1) // GROUP_ROWS  # 25 (last group 212 rows)
C_ZG = 64  # how many Z_g tiles stay cached in SBUF (rest re-read in stage 2)

f16 = mybir.dt.float16
f32 = mybir.dt.float32


def _emit_body(nc, pools, tensors, c_zg):
    add = mybir.AluOpType.add
    mult = mybir.AluOpType.mult
    AF = mybir.ActivationFunctionType
    fix, s1, zop, otp, smp, psfix, psd, dram = pools
    (zl_d, zg_d, zoT_d, w_d, b_d, out_d, consts, zd_cache, zg_cache) = tensors
    ones, nones, one11_16, ones_row, one11_32 = consts

    # ---------------- Stage 1: colsum + cache fill ----------------
    ps_s = psfix.tile([1, CH], f32, tag="ps_s")
    for i in range(N_TILES):
        r0 = i * P
        R = min(P, SHARD - r0)
        zlb = s1.tile([P, CH], f16, tag="zl")
        nc.sync.dma_start(zlb[:R], zl_d[r0 : r0 + R])
        if i < c_zg:
            zgb, zc0 = zg_cache, i * CH
        else:
            zgb = s1.tile([P, CH], f16, tag="zg", name="zgt1")
            zc0 = 0
        nc.sync.dma_start(zgb[:R, zc0 : zc0 + CH], zg_d[r0 : r0 + R])
        nc.tensor.matmul(ps_s[:], ones[:R], zlb[:R], start=(i == 0), stop=False)
        nc.tensor.matmul(
            ps_s[:], nones[:R], zgb[:R, zc0 : zc0 + CH],
            start=False, stop=(i == N_TILES - 1),
        )
        nc.vector.tensor_sub(
            zd_cache[:R, i * CH : (i + 1) * CH], zlb[:R], zgb[:R, zc0 : zc0 + CH]
        )

    # ---------------- AllReduce of s ----------------
    s_sb = fix.tile([1, CH], f32, tag="s_sb")
    nc.vector.tensor_copy(s_sb[:], ps_s[:])
    s_part = dram.tile([1, CH], f32, tag="s_part")
    s_glob = dram.tile([1, CH], f32, tag="s_glob")
    nc.sync.dma_start(s_part[:, :], s_sb[:])
    nc.gpsimd.collective_compute(
        "AllReduce",
        add,
        replica_groups=[list(range(N_CORES))],
        ins=[s_part.opt()],
        outs=[s_glob.opt()],
    )
    s_all = fix.tile([1, CH], f32, tag="s_all")
    nc.sync.dma_start(s_all[:], s_glob[:, :])

    # ---------------- Interlude: u, c on device ----------------
    ps_u = psfix.tile([1, CH], f32, tag="ps_u")
    ps_c = psfix.tile([1, 1], f32, tag="ps_c")
    scks = []
    for k in range(4):
        ps_sc = psd.tile([P, 1], f32, tag="tr", bufs=1)
        nc.tensor.matmul(
            ps_sc[:], s_all[0:1, k * P : (k + 1) * P], one11_32[:],
            start=True, stop=True,
        )
        sck = fix.tile([P, 1], f32, tag=f"sck{k}")
        nc.vector.tensor_copy(sck[:], ps_sc[:])
        scks.append(sck)
    for k in range(4):
        wk = s1.tile([P, CH], f32, tag="wk")
        nc.sync.dma_start(wk[:], w_d[k * P : (k + 1) * P])
        nc.tensor.matmul(ps_u[:], scks[k][:], wk[:], start=(k == 0), stop=(k == 3))
    for k in range(4):
        bk = fix.tile([P, 1], f32, tag=f"bk{k}")
        nc.sync.dma_start(bk[:], b_d[k * P : (k + 1) * P, 0:1])
        nc.tensor.matmul(ps_c[:], scks[k][:], bk[:], start=(k == 0), stop=(k == 3))
    c_sb = fix.tile([1, 1], f32, tag="c_sb")
    nc.vector.tensor_copy(c_sb[:], ps_c[:])
    ps_cb = psd.tile([P, 1], f32, tag="tr", bufs=1)
    nc.tensor.matmul(ps_cb[:], ones_row[:], c_sb[:], start=True, stop=True)
    c_b = fix.tile([P, 1], f32, tag="c_b")
    nc.vector.tensor_copy(c_b[:], ps_cb[:])

    u_sb = fix.tile([1, CH], f32, tag="u_sb")
    nc.vector.tensor_copy(u_sb[:], ps_u[:])
    u_hi = fix.tile([1, CH], f16, tag="u_hi")
    nc.vector.tensor_copy(u_hi[:], u_sb[:])
    u_hi32 = fix.tile([1, CH], f32, tag="u_hi32")
    nc.vector.tensor_copy(u_hi32[:], u_hi[:])
    u_lo = fix.tile([1, CH], f16, tag="u_lo")
    nc.vector.tensor_sub(u_lo[:], u_sb[:], u_hi32[:])
    u2 = []
    for k in range(4):
        u2k = fix.tile([P, 2], f16, tag=f"u2_{k}")
        for h, src in enumerate((u_hi, u_lo)):
            ps_tr = psd.tile([P, 1], f32, tag="tr", bufs=1)
            nc.tensor.matmul(
                ps_tr[:], src[0:1, k * P : (k + 1) * P], one11_16[:],
                start=True, stop=True,
            )
            nc.vector.tensor_copy(u2k[:, h : h + 1], ps_tr[:])
        u2.append(u2k)

    # ---------------- Stage 2: d, sigmoid, blend ----------------
    for g in range(N_GROUPS):
        c0 = g * GROUP_ROWS
        GW = min(GROUP_ROWS, SHARD - c0)
        zot = []
        for k in range(4):
            zt = zop.tile([P, GROUP_ROWS], f16, tag=f"zo{k}")
            nc.sync.dma_start(zt[:, :GW], zoT_d[k * P : (k + 1) * P, c0 : c0 + GW])
            zot.append(zt)
        for j in range((GW + P - 1) // P):
            i = g * (GROUP_ROWS // P) + j
            r0 = c0 + j * P
            R = min(P, SHARD - r0)
            ps_d = psd.tile([P, 1], f32, tag="d", bufs=3)
            for k in range(4):
                nc.tensor.matmul(
                    ps_d[:R], zot[k][:, j * P : j * P + R], u2[k][:, 0:1],
                    start=(k == 0), stop=False,
                )
                nc.tensor.matmul(
                    ps_d[:R], zot[k][:, j * P : j * P + R], u2[k][:, 1:2],
                    start=False, stop=(k == 3),
                )
            p_t = smp.tile([P, 1], f32, tag="p")
            nc.scalar.activation(
                p_t[:R], ps_d[:R], AF.Sigmoid, bias=c_b[:R, 0:1], scale=1.0
            )
            if i < c_zg:
                zgb, zc0 = zg_cache, i * CH
            else:
                zgb = s1.tile([P, CH], f16, tag="zg2", name="zgt2")
                zc0 = 0
                nc.sync.dma_start(zgb[:R, 0:CH], zg_d[r0 : r0 + R])
            ot = otp.tile([P, CH], f16, tag="ot")
            nc.vector.scalar_tensor_tensor(
                ot[:R],
                zd_cache[:R, i * CH : (i + 1) * CH],
                p_t[:R, 0:1],
                zgb[:R, zc0 : zc0 + CH],
                op0=mult,
                op1=add,
            )
            nc.sync.dma_start(out_d[r0 : r0 + R], ot[:R])


def build_nc(c_zg=C_ZG, bufs=4, rep_loop=1, rep_mode="hw"):
    import contextlib

    nc = bacc.Bacc(
        "TRN2",
        target_bir_lowering=False,
        debug=False,
        enable_asserts=False,
        num_devices=N_CORES,
    )
    zl_d = nc.dram_tensor("Z_l", [SHARD, CH], f16, kind="ExternalInput")
    zg_d = nc.dram_tensor("Z_g", [SHARD, CH], f16, kind="ExternalInput")
    zoT_d = nc.dram_tensor("ZoT", [CH, SHARD], f16, kind="ExternalInput")
    w_d = nc.dram_tensor("W", [CH, CH], f32, kind="ExternalInput")
    b_d = nc.dram_tensor("b", [CH, 1], f32, kind="ExternalInput")
    out_d = nc.dram_tensor("out", [SHARD, CH], f16, kind="ExternalOutput")

    with tile.TileContext(nc) as tc:
        with (
            tc.tile_pool(name="cache", bufs=1) as cache,
            tc.tile_pool(name="fix", bufs=1) as fix,
            tc.tile_pool(name="s1", bufs=bufs) as s1,
            tc.tile_pool(name="zo", bufs=2) as zop,
            tc.tile_pool(name="ot", bufs=bufs) as otp,
            tc.tile_pool(name="sm", bufs=bufs) as smp,
            tc.tile_pool(name="psfix", bufs=1, space="PSUM") as psfix,
            tc.tile_pool(name="psd", bufs=4, space="PSUM") as psd,
            tc.tile_pool(name="dram", bufs=1, space="DRAM") as dram,
        ):
            ones = fix.tile([P, 1], f16, tag="ones")
            nones = fix.tile([P, 1], f16, tag="nones")
            one11_16 = fix.tile([1, 1], f16, tag="one11_16")
            ones_row = fix.tile([1, P], f32, tag="ones_row")
            one11_32 = fix.tile([1, 1], f32, tag="one11_32")
            nc.vector.memset(ones[:], 1.0)
            nc.vector.memset(nones[:], -1.0)
            nc.vector.memset(one11_16[:], 1.0)
            nc.vector.memset(ones_row[:], 1.0)
            nc.vector.memset(one11_32[:], 1.0)
            consts = (ones, nones, one11_16, ones_row, one11_32)

            zd_cache = cache.tile([P, N_TILES * CH], f16, tag="zd")
            zg_cache = cache.tile([P, c_zg * CH], f16, tag="zg")

            pools = (fix, s1, zop, otp, smp, psfix, psd, dram)
            tensors = (
                zl_d, zg_d, zoT_d, w_d, b_d, out_d, consts, zd_cache, zg_cache
            )
            if rep_loop > 1 and rep_mode == "unroll":
                for _ in range(rep_loop):
                    _emit_body(nc, pools, tensors, c_zg)
            else:
                rep_ctx = (
                    tc.For_i(0, rep_loop, 1)
                    if rep_loop > 1
                    else contextlib.nullcontext()
                )
                with rep_ctx:
                    _emit_body(nc, pools, tensors, c_zg)
    nc.compile()
    return nc


_CACHE = {}


def _prep_maps(Z_o, Z_l, Z_g, W, b):
    W32 = np.ascontiguousarray(np.asarray(W, dtype=np.float32))
    b32 = np.ascontiguousarray(np.asarray(b, dtype=np.float32).reshape(CH, 1))
    maps = []
    for i in range(N_CORES):
        sl = slice(i * SHARD, (i + 1) * SHARD)
        zo16 = np.asarray(Z_o[sl], dtype=np.float16)
        maps.append(
            {
                "Z_l": np.ascontiguousarray(np.asarray(Z_l[sl], dtype=np.float16)),
                "Z_g": np.ascontiguousarray(np.asarray(Z_g[sl], dtype=np.float16)),
                "ZoT": np.ascontiguousarray(zo16.T),
                "W": W32,
                "b": b32,
            }
        )
    return maps


def kernel(Z_o, Z_l, Z_g, W, b):
    if "nc" not in _CACHE:
        _CACHE["nc"] = build_nc()
    nc = _CACHE["nc"]
    maps = _prep_maps(Z_o, Z_l, Z_g, W, b)
    res = run_bass_kernel_spmd(nc, maps, core_ids=list(range(N_CORES)))
    out = np.concatenate([r["out"] for r in res.results], axis=0)
    return out.astype(np.float32)


# revision 8
# speedup vs baseline: 25.2457x; 5.0888x over previous
"""Trainium2 Bass kernel for nn_Attention2 (attention-gated blend of Z_l/Z_g).

Reference math:
    Q     = Z_o @ W.T + b                      # [N, 512]
    att_l = Q @ colsum(Z_l)                    # [N]
    att_g = Q @ colsum(Z_g)                    # [N]
    att   = softmax([att_l, att_g], axis=1)    # [N, 2]
    out   = Z_l * att[:, 0:1] + Z_g * att[:, 1:2]

Only d = att_l - att_g matters (2-way softmax == sigmoid), and it folds:
    s = colsum(Z_l) - colsum(Z_g)              # [512]
    u = W.T @ s                                # [512]
    c = b . s                                  # scalar
    d = Z_o @ u + c                            # [N]
    out = Z_g + sigmoid(d) * (Z_l - Z_g)

Fused single-launch SPMD design (8 cores, rows sharded, fp16 I/O):
  Stage 1: stream Z_l/Z_g in 4-tile (512-row) merged DMAs; accumulate
    s_partial in PSUM via TensorE (+1/-1 ones matmuls); cache zd = Z_l - Z_g
    (all tiles) and Z_g (most tiles) in SBUF.
  AllReduce(add) of s_partial [1,512] f32 across the 8 cores (DRAM bounce).
  Interlude: u = W.T @ s via 4 PSUM matmuls; split u into fp16 (hi, lo) pair
    so the stage-2 fp16 matmul keeps f32 accuracy of u; c = b.s broadcast.
  Stage 2: stream Z_o TRANSPOSED (fp16 [512, shard]) one 512-row group per
    DMA (all 4 channel chunks in one 3D access pattern); per 128-row tile
    d accumulates in PSUM [R,1] via 8 matmuls (4 chunks x u_hi/u_lo);
    p = sigmoid(d + c) on ScalarE (reads PSUM); out = (zd * p) + zg in one
    DVE scalar_tensor_tensor into a 4-tile staging buffer; one merged DMA
    writes the group (fp16).
  DMA issue is split across both HWDGE rings: Z_l/Z_g on SyncE, ZoT/out on
  ScalarE (each dma_start costs ~650-780ns of issue time on its engine).
Host: downcast/transpose inputs, upcast output; tiny work only.

Precision: fp16 inputs give rel err ~8e-3 vs the f32 reference (gate 2e-2).
"""

import numpy as np

import concourse.bacc as bacc
import concourse.mybir as mybir
import concourse.tile as tile
from concourse.bass_types import AP
from concourse.bass_utils import run_bass_kernel_spmd

N_CORES = 8
N_TOTAL = 100000
CH = 512
SHARD = N_TOTAL // N_CORES  # 12500
P = 128
N_TILES = (SHARD + P - 1) // P  # 98 (last tile 84 rows)
N_FULL_CHUNKS = 24  # chunks of 4 full tiles; tiles 96, 97 handled singly
C_ZG = 56  # Z_g tiles cached in SBUF (multiple of 4); rest re-read in stage 2

f16 = mybir.dt.float16
f32 = mybir.dt.float32


def _rows_ap(dram_t, r0, nseg):
    """[P, nseg, CH] view of nseg row-tiles: partition p, seg s -> row r0+s*P+p."""
    h = dram_t[0:1].tensor
    return AP(h, r0 * CH, [[CH, P], [P * CH, nseg], [1, CH]])


def _zot_ap(dram_t, c0, gw):
    """[P, 4, gw] view: partition p, seg k, col r -> zoT[k*P+p, c0+r]."""
    h = dram_t[0:1].tensor
    return AP(h, c0, [[SHARD, P], [P * SHARD, 4], [1, gw]])


def _emit_body(nc, pools, tensors, c_zg):
    add = mybir.AluOpType.add
    mult = mybir.AluOpType.mult
    AF = mybir.ActivationFunctionType
    fix, s1, zop, otp, smp, psfix, psd, dram = pools
    (zl_d, zg_d, zoT_d, w_d, b_d, out_d, consts, zd_cache, zg_cache) = tensors
    ones, nones, one11_16, ones_row, one11_32 = consts
    n_zg_chunks = c_zg // 4

    # ---------------- Stage 1: colsum + cache fill ----------------
    ps_s = psfix.tile([1, CH], f32, tag="ps_s")

    def colsum_pair(zl_t, zl_c0, zg_t, zg_c0, R, first, last):
        nc.tensor.matmul(
            ps_s[:], ones[:R], zl_t[:R, zl_c0 : zl_c0 + CH],
            start=first, stop=False,
        )
        nc.tensor.matmul(
            ps_s[:], nones[:R], zg_t[:R, zg_c0 : zg_c0 + CH],
            start=False, stop=last,
        )

    for ci in range(N_FULL_CHUNKS):
        r0 = ci * 4 * P
        zl4 = s1.tile([P, 4 * CH], f16, tag="zl4", bufs=2)
        nc.sync.dma_start(zl4[:, :], _rows_ap(zl_d, r0, 4))
        if ci < n_zg_chunks:
            zg4, zgc0 = zg_cache, ci * 4 * CH
        else:
            zg4 = s1.tile([P, 4 * CH], f16, tag="zgr", bufs=2, name="zg4t")
            zgc0 = 0
        nc.sync.dma_start(zg4[:, zgc0 : zgc0 + 4 * CH], _rows_ap(zg_d, r0, 4))
        for t in range(4):
            colsum_pair(
                zl4, t * CH, zg4, zgc0 + t * CH, P, ci == 0 and t == 0, False
            )
        nc.vector.tensor_sub(
            zd_cache[:, ci * 4 * CH : (ci + 1) * 4 * CH],
            zl4[:, :],
            zg4[:, zgc0 : zgc0 + 4 * CH],
        )
    # tail tiles 96, 97
    for i in (96, 97):
        r0 = i * P
        R = min(P, SHARD - r0)
        zlt = s1.tile([P, CH], f16, tag="zlt", bufs=2)
        nc.sync.dma_start(zlt[:R], zl_d[r0 : r0 + R])
        zgt = s1.tile([P, CH], f16, tag="zgt", bufs=2)
        nc.sync.dma_start(zgt[:R], zg_d[r0 : r0 + R])
        colsum_pair(zlt, 0, zgt, 0, R, False, i == 97)
        nc.vector.tensor_sub(
            zd_cache[:R, i * CH : (i + 1) * CH], zlt[:R], zgt[:R]
        )

    # ---------------- AllReduce of s ----------------
    s_sb = fix.tile([1, CH], f32, tag="s_sb")
    nc.vector.tensor_copy(s_sb[:], ps_s[:])
    s_part = dram.tile([1, CH], f32, tag="s_part")
    s_glob = dram.tile([1, CH], f32, tag="s_glob")
    nc.sync.dma_start(s_part[:, :], s_sb[:])
    nc.gpsimd.collective_compute(
        "AllReduce",
        add,
        replica_groups=[list(range(N_CORES))],
        ins=[s_part.opt()],
        outs=[s_glob.opt()],
    )
    s_all = fix.tile([1, CH], f32, tag="s_all")
    nc.sync.dma_start(s_all[:], s_glob[:, :])

    # ---------------- Interlude: u, c on device ----------------
    ps_u = psfix.tile([1, CH], f32, tag="ps_u")
    ps_c = psfix.tile([1, 1], f32, tag="ps_c")
    scks = []
    for k in range(4):
        ps_sc = psd.tile([P, 1], f32, tag="tr", bufs=1)
        nc.tensor.matmul(
            ps_sc[:], s_all[0:1, k * P : (k + 1) * P], one11_32[:],
            start=True, stop=True,
        )
        sck = fix.tile([P, 1], f32, tag=f"sck{k}")
        nc.vector.tensor_copy(sck[:], ps_sc[:])
        scks.append(sck)
    for k in range(4):
        wk = s1.tile([P, CH], f32, tag="wk", bufs=2)
        nc.sync.dma_start(wk[:], w_d[k * P : (k + 1) * P])
        nc.tensor.matmul(ps_u[:], scks[k][:], wk[:], start=(k == 0), stop=(k == 3))
    for k in range(4):
        bk = fix.tile([P, 1], f32, tag=f"bk{k}")
        nc.sync.dma_start(bk[:], b_d[k * P : (k + 1) * P, 0:1])
        nc.tensor.matmul(ps_c[:], scks[k][:], bk[:], start=(k == 0), stop=(k == 3))
    c_sb = fix.tile([1, 1], f32, tag="c_sb")
    nc.vector.tensor_copy(c_sb[:], ps_c[:])
    ps_cb = psd.tile([P, 1], f32, tag="tr", bufs=1)
    nc.tensor.matmul(ps_cb[:], ones_row[:], c_sb[:], start=True, stop=True)
    c_b = fix.tile([P, 1], f32, tag="c_b")
    nc.vector.tensor_copy(c_b[:], ps_cb[:])

    u_sb = fix.tile([1, CH], f32, tag="u_sb")
    nc.vector.tensor_copy(u_sb[:], ps_u[:])
    u_hi = fix.tile([1, CH], f16, tag="u_hi")
    nc.vector.tensor_copy(u_hi[:], u_sb[:])
    u_hi32 = fix.tile([1, CH], f32, tag="u_hi32")
    nc.vector.tensor_copy(u_hi32[:], u_hi[:])
    u_lo = fix.tile([1, CH], f16, tag="u_lo")
    nc.vector.tensor_sub(u_lo[:], u_sb[:], u_hi32[:])
    u2 = []
    for k in range(4):
        u2k = fix.tile([P, 2], f16, tag=f"u2_{k}")
        for h, src in enumerate((u_hi, u_lo)):
            ps_tr = psd.tile([P, 1], f32, tag="tr", bufs=1)
            nc.tensor.matmul(
                ps_tr[:], src[0:1, k * P : (k + 1) * P], one11_16[:],
                start=True, stop=True,
            )
            nc.vector.tensor_copy(u2k[:, h : h + 1], ps_tr[:])
        u2.append(u2k)

    # ---------------- Stage 2: d, sigmoid, blend ----------------
    for g in range(N_FULL_CHUNKS + 1):
        c0 = g * 4 * P
        GW = min(4 * P, SHARD - c0)
        n_t = (GW + P - 1) // P  # 4, or 2 in the last group
        zot = zop.tile([P, 4 * 4 * P], f16, tag="zot", bufs=2)
        nc.scalar.dma_start(zot[:, : 4 * GW], _zot_ap(zoT_d, c0, GW))
        need_zg_dma = g >= n_zg_chunks
        if need_zg_dma:
            zgr = s1.tile([P, 4 * CH], f16, tag="zgr", bufs=2)
            if GW == 4 * P:
                nc.sync.dma_start(zgr[:, :], _rows_ap(zg_d, c0, 4))
            else:
                for t in range(n_t):
                    r0 = c0 + t * P
                    R = min(P, SHARD - r0)
                    nc.sync.dma_start(
                        zgr[:R, t * CH : t * CH + CH], zg_d[r0 : r0 + R]
                    )
        outm = otp.tile([P, 4 * CH], f16, tag="outm", bufs=2)
        for j in range(n_t):
            i = g * 4 + j
            r0 = c0 + j * P
            R = min(P, SHARD - r0)
            ps_d = psd.tile([P, 1], f32, tag="d", bufs=3)
            for k in range(4):
                lhs = zot[:, k * GW + j * P : k * GW + j * P + R]
                nc.tensor.matmul(
                    ps_d[:R], lhs, u2[k][:, 0:1], start=(k == 0), stop=False
                )
                nc.tensor.matmul(
                    ps_d[:R], lhs, u2[k][:, 1:2], start=False, stop=(k == 3)
                )
            p_t = smp.tile([P, 1], f32, tag="p")
            nc.scalar.activation(
                p_t[:R], ps_d[:R], AF.Sigmoid, bias=c_b[:R, 0:1], scale=1.0
            )
            if i < c_zg:
                zgb, zc0 = zg_cache, i * CH
            else:
                zgb, zc0 = zgr, j * CH
            nc.vector.scalar_tensor_tensor(
                outm[:R, j * CH : (j + 1) * CH],
                zd_cache[:R, i * CH : (i + 1) * CH],
                p_t[:R, 0:1],
                zgb[:R, zc0 : zc0 + CH],
                op0=mult,
                op1=add,
            )
        if GW == 4 * P:
            nc.scalar.dma_start(_rows_ap(out_d, c0, 4), outm[:, :])
        else:
            for t in range(n_t):
                r0 = c0 + t * P
                R = min(P, SHARD - r0)
                nc.scalar.dma_start(
                    out_d[r0 : r0 + R], outm[:R, t * CH : t * CH + CH]
                )


def build_nc(c_zg=C_ZG, bufs=4, rep_loop=1, rep_mode="unroll"):
    import contextlib

    nc = bacc.Bacc(
        "TRN2",
        target_bir_lowering=False,
        debug=False,
        enable_asserts=False,
        num_devices=N_CORES,
    )
    zl_d = nc.dram_tensor("Z_l", [SHARD, CH], f16, kind="ExternalInput")
    zg_d = nc.dram_tensor("Z_g", [SHARD, CH], f16, kind="ExternalInput")
    zoT_d = nc.dram_tensor("ZoT", [CH, SHARD], f16, kind="ExternalInput")
    w_d = nc.dram_tensor("W", [CH, CH], f32, kind="ExternalInput")
    b_d = nc.dram_tensor("b", [CH, 1], f32, kind="ExternalInput")
    out_d = nc.dram_tensor("out", [SHARD, CH], f16, kind="ExternalOutput")

    with tile.TileContext(nc) as tc:
        with (
            tc.tile_pool(name="cache", bufs=1) as cache,
            tc.tile_pool(name="fix", bufs=1) as fix,
            tc.tile_pool(name="s1", bufs=2) as s1,
            tc.tile_pool(name="zo", bufs=2) as zop,
            tc.tile_pool(name="ot", bufs=2) as otp,
            tc.tile_pool(name="sm", bufs=4) as smp,
            tc.tile_pool(name="psfix", bufs=1, space="PSUM") as psfix,
            tc.tile_pool(name="psd", bufs=4, space="PSUM") as psd,
            tc.tile_pool(name="dram", bufs=1, space="DRAM") as dram,
        ):
            ones = fix.tile([P, 1], f16, tag="ones")
            nones = fix.tile([P, 1], f16, tag="nones")
            one11_16 = fix.tile([1, 1], f16, tag="one11_16")
            ones_row = fix.tile([1, P], f32, tag="ones_row")
            one11_32 = fix.tile([1, 1], f32, tag="one11_32")
            nc.vector.memset(ones[:], 1.0)
            nc.vector.memset(nones[:], -1.0)
            nc.vector.memset(one11_16[:], 1.0)
            nc.vector.memset(ones_row[:], 1.0)
            nc.vector.memset(one11_32[:], 1.0)
            consts = (ones, nones, one11_16, ones_row, one11_32)

            zd_cache = cache.tile([P, N_TILES * CH], f16, tag="zd")
            zg_cache = cache.tile([P, c_zg * CH], f16, tag="zg")

            pools = (fix, s1, zop, otp, smp, psfix, psd, dram)
            tensors = (
                zl_d, zg_d, zoT_d, w_d, b_d, out_d, consts, zd_cache, zg_cache
            )
            if rep_loop > 1 and rep_mode == "unroll":
                for _ in range(rep_loop):
                    _emit_body(nc, pools, tensors, c_zg)
            else:
                rep_ctx = (
                    tc.For_i(0, rep_loop, 1)
                    if rep_loop > 1
                    else contextlib.nullcontext()
                )
                with rep_ctx:
                    _emit_body(nc, pools, tensors, c_zg)
    nc.compile()
    return nc


_CACHE = {}


def _prep_maps(Z_o, Z_l, Z_g, W, b):
    W32 = np.ascontiguousarray(np.asarray(W, dtype=np.float32))
    b32 = np.ascontiguousarray(np.asarray(b, dtype=np.float32).reshape(CH, 1))
    maps = []
    for i in range(N_CORES):
        sl = slice(i * SHARD, (i + 1) * SHARD)
        zo16 = np.asarray(Z_o[sl], dtype=np.float16)
        maps.append(
            {
                "Z_l": np.ascontiguousarray(np.asarray(Z_l[sl], dtype=np.float16)),
                "Z_g": np.ascontiguousarray(np.asarray(Z_g[sl], dtype=np.float16)),
                "ZoT": np.ascontiguousarray(zo16.T),
                "W": W32,
                "b": b32,
            }
        )
    return maps


def kernel(Z_o, Z_l, Z_g, W, b):
    if "nc" not in _CACHE:
        _CACHE["nc"] = build_nc()
    nc = _CACHE["nc"]
    maps = _prep_maps(Z_o, Z_l, Z_g, W, b)
    res = run_bass_kernel_spmd(nc, maps, core_ids=list(range(N_CORES)))
    out = np.concatenate([r["out"] for r in res.results], axis=0)
    return out.astype(np.float32)
